# revision 3
# baseline (speedup 1.0000x reference)
"""V4 Trainium2 kernel for nn_EncoderTreeSpanNN — paired gathers + lean tensor path.

Changes vs V3:
- Pair trick: each span's 8 tokens are grouped into 4 same-partition pairs
  (pair types LL/LH/HH chosen per span so the global lo/hi slot budgets are
  met exactly — zero padding, same 9.4MB/core gather traffic). Two DVE adds
  fold the 8 gather blocks to 4 summed blocks, HALVING the span-reduction
  matmuls (96 -> 48 per core) and the selection-matrix size.
- Row-sums are folded into the output matmul: kf tiles are 129 wide with a
  ones column, so each [128,129] PSUM region carries sum(p*kf) and sum(p)
  together. The 48 rsum matmuls are gone.
- One gather tile per group (12, no pool recycling): gathers never throttle
  on compute consuming earlier groups.
- Selection matrices are all built during the GPSIMD-library-load dead
  window (~6-16us) when DVE is otherwise idle.
- idx upload is issued first so desc-gen can start the moment the library
  overlay lands.
"""

import sys

sys.path.insert(0, "/opt/trn_rl_repo")

import numpy as np

import concourse.bacc as bacc
import concourse.tile as tile
from concourse import mybir
from concourse.bass_utils import run_bass_kernel_spmd

# problem constants
V, D, HOPS = 50000, 128, 3
B, Lc, Mc = 16, 256, 8
Lk, Mk = 512, 8
NCORES = 8
BPC = B // NCORES
E3 = HOPS * D  # 384 elems per row (hop-interleaved)
LO_MAX = 32768  # lo view = tab[0:32768], idx = t (int16 max)
HI_BASE = 17232  # hi view = tab[17232:50000], idx = t-17232 (max 32767)
CONV_G = Lc // 128  # 2
KB_G = Lk // 128  # 4
NG_PER_B = CONV_G + KB_G  # 6
NG = BPC * NG_PER_B  # 12
PB = 4  # pair-blocks per group
# pair-block -> (first gather block, second gather block); blocks 0-3 are the
# lo gather, 4-7 the hi gather. Sum lands in the first block of each pair.
PAIR_SRC = [(0, 1), (2, 4), (3, 5), (6, 7)]  # LL, LH, LH, HH
SUM_BLOCKS = [0, 2, 3, 6]

F32 = mybir.dt.float32
F16 = mybir.dt.float16
I16 = mybir.dt.int16

EXP_BIAS = float(-4.0 * np.log(2.0))  # exp(att)*2^-4 keeps f16 range

# per-core group list: per batch, conv groups then kb groups
GROUPS = []
for _b in range(BPC):
    for _gg in range(CONV_G):
        GROUPS.append(("c", _b, _gg))
    for _gg in range(KB_G):
        GROUPS.append(("k", _b, _gg))


def _pack_idx(flat):
    """[n] int16 -> [128, n//16] dma_gather index layout (8 replicas x 16)."""
    n = flat.shape[0]
    return np.tile(flat.reshape(n // 16, 16).T.astype(np.int16), (8, 1))


def _plan_group(toks):
    """toks [128, 8] -> per-slot token/segment layout for one group.

    Returns (idx_lo [512] int16, idx_hi [512] int16, seg [128, 4] f32).
    Position p*1 within block b is partition p; lo positions b*128+p for
    b in 0..3, hi likewise. Every slot is filled (no padding): span k's 8
    tokens form 4 pairs typed LL/LH/HH so each pair's two tokens share a
    partition across its pair's two gather blocks.
    """
    c = (toks < HI_BASE).sum(1)
    d = (toks >= LO_MAX).sum(1)
    f = 8 - c - d
    need = 512 - int(c.sum())
    assert 0 <= need <= int(f.sum()), "lo/hi flex band cannot balance group"
    cumf = np.cumsum(f) - f
    add = np.clip(need - cumf, 0, f)
    cp = c + add  # tokens sent to the lo view, per span
    ymin = cp % 2
    cap = (np.minimum(cp, 8 - cp) - ymin) // 2
    bumps = 256 - int(ymin.sum())
    assert bumps >= 0 and bumps % 2 == 0
    bumps //= 2
    assert cap.sum() >= bumps, "cannot meet LH pair budget"
    cumcap = np.cumsum(cap) - cap
    y = ymin + 2 * np.clip(bumps - cumcap, 0, cap)  # LH pairs per span
    x = (cp - y) // 2  # LL pairs
    z = (8 - cp - y) // 2  # HH pairs
    assert x.sum() == 128 and y.sum() == 256 and z.sum() == 128

    idx_lo = np.empty(512, np.int16)
    idx_hi = np.empty(512, np.int16)
    seg = np.empty((128, PB), np.float32)
    ll = lh = hh = 0
    for k in range(128):
        t = toks[k]
        lo_mask = t < HI_BASE
        flex_mask = (t >= HI_BASE) & (t < LO_MAX)
        lo_list = list(t[lo_mask])
        flex_list = list(t[flex_mask])
        hi_list = list(t[t >= LO_MAX])
        extra = int(cp[k]) - len(lo_list)
        lo_list += flex_list[:extra]
        hi_list += flex_list[extra:]
        assert len(lo_list) == cp[k] and len(hi_list) == 8 - cp[k]
        # y LH pairs first, then LL from remaining lo, HH from remaining hi
        for _ in range(int(y[k])):
            a, bb = lo_list.pop(), hi_list.pop()
            p, slot = (lh, 1) if lh < 128 else (lh - 128, 2)
            lh += 1
            idx_lo[(2 + (slot - 1)) * 128 + p] = a
            idx_hi[(slot - 1) * 128 + p] = bb - HI_BASE
            seg[p, slot] = k
        for _ in range(int(x[k])):
            a, bb = lo_list.pop(), lo_list.pop()
            idx_lo[0 * 128 + ll] = a
            idx_lo[1 * 128 + ll] = bb
            seg[ll, 0] = k
            ll += 1
        for _ in range(int(z[k])):
            a, bb = hi_list.pop(), hi_list.pop()
            idx_hi[2 * 128 + hh] = a - HI_BASE
            idx_hi[3 * 128 + hh] = bb - HI_BASE
            seg[hh, 3] = k
            hh += 1
        assert not lo_list and not hi_list
    assert ll == 128 and lh == 256 and hh == 128
    return idx_lo, idx_hi, seg


def prepare(conv_seqs, kb_arr, C, K):
    conv_seqs = np.asarray(conv_seqs)
    kb_arr = np.asarray(kb_arr)

    def row_table(T):
        # [HOPS, V, D] -> [V, HOPS*D] f16 (hop-interleaved rows)
        return (
            np.transpose(np.asarray(T, np.float32), (1, 0, 2))
            .reshape(V, E3)
            .astype(np.float16)
        )

    tab_c = row_table(C)
    tab_k = row_table(K)

    in_maps = []
    for cix in range(NCORES):
        idx_all = np.empty((128, NG * 2 * 32), np.int16)
        seg_all = np.empty((128, NG, PB), np.float32)
        for g, (t, b, gg) in enumerate(GROUPS):
            seqs = conv_seqs if t == "c" else kb_arr
            toks = seqs[cix * BPC + b, gg * 128 : (gg + 1) * 128, :].astype(np.int64)
            idx_lo, idx_hi, seg = _plan_group(toks)
            idx_all[:, (g * 2) * 32 : (g * 2 + 1) * 32] = _pack_idx(idx_lo)
            idx_all[:, (g * 2 + 1) * 32 : (g * 2 + 2) * 32] = _pack_idx(idx_hi)
            seg_all[:, g, :] = seg
        in_maps.append(
            {
                "tab_c": tab_c,
                "tab_k": tab_k,
                "idx_all": idx_all,
                "seg_all": seg_all,
                "ident": np.eye(128, dtype=np.float16),
                "iota": np.broadcast_to(
                    np.arange(128, dtype=np.float32), (128, 128)
                ).copy(),
            }
        )
    return {}, in_maps


def build_nc(meta):
    nc = bacc.Bacc(num_swdge_queues=4)
    tab_c = nc.declare_dram_parameter("tab_c", [V, E3], F16, False)
    tab_k = nc.declare_dram_parameter("tab_k", [V, E3], F16, False)
    idx_d = nc.declare_dram_parameter("idx_all", [128, NG * 2 * 32], I16, False)
    seg_d = nc.declare_dram_parameter("seg_all", [128, NG, PB], F32, False)
    ident_d = nc.declare_dram_parameter("ident", [128, 128], F16, False)
    iota_d = nc.declare_dram_parameter("iota", [128, 128], F32, False)
    out_d = nc.declare_dram_parameter("out", [BPC, Lc, D], F32, True)

    tab_lo = {"c": tab_c[0:LO_MAX], "k": tab_k[0:LO_MAX]}
    tab_hi = {"c": tab_c[HI_BASE:V], "k": tab_k[HI_BASE:V]}

    with tile.TileContext(nc) as tc:
        with (
            tc.tile_pool(name="constp", bufs=1) as constp,
            tc.tile_pool(name="gp", bufs=1) as gp,
            tc.tile_pool(name="sp", bufs=1) as sp,
            tc.tile_pool(name="featp", bufs=1) as featp,
            tc.tile_pool(name="convscr", bufs=2) as convscr,
            tc.tile_pool(name="expp", bufs=3) as expp,
            tc.tile_pool(name="softp", bufs=2) as softp,
            tc.tile_pool(name="outps_p", bufs=1, space="PSUM") as outps_p,
            tc.tile_pool(name="spanps_p", bufs=2, space="PSUM") as spanps_p,
            tc.tile_pool(name="tp_p", bufs=2, space="PSUM") as tp_p,
            tc.tile_pool(name="attps_p", bufs=2, space="PSUM") as attps_p,
        ):
            # tiny dummy gather first: forces the GPSIMD library load to start
            # as early as the engine allows
            dummy_idx = constp.tile([128, 1], I16)
            nc.vector.memset(dummy_idx[:], 0)
            dummy_out = constp.tile([128, 1, E3], F16)
            nc.gpsimd.dma_gather(
                out_ap=dummy_out[:],
                in_ap=tab_c[0:LO_MAX],
                idxs_ap=dummy_idx[:],
                num_idxs=16,
                num_idxs_reg=16,
                elem_size=E3,
                queue_num=0,
            )
            idx_sb = constp.tile([128, NG * 2 * 32], I16)
            nc.sync.dma_start(out=idx_sb[:], in_=idx_d[:])
            seg_sb = constp.tile([128, NG, PB], F32)
            nc.sync.dma_start(out=seg_sb[:], in_=seg_d[:])
            ident = constp.tile([128, 128], F16)
            nc.sync.dma_start(out=ident[:], in_=ident_d[:])
            iota = constp.tile([128, 128], F32)
            nc.sync.dma_start(out=iota[:], in_=iota_d[:])
            ebias = constp.tile([128, 1], F32)
            nc.vector.memset(ebias[:], EXP_BIAS)

            # per-batch persistent feature tiles
            kf3 = [
                featp.tile([128, KB_G, HOPS, 129], F16, name=f"kf3_{b}")
                for b in range(BPC)
            ]
            cfT3 = [
                featp.tile([128, HOPS, Lc], F16, name=f"cfT3_{b}") for b in range(BPC)
            ]
            kfT3 = [
                featp.tile([128, HOPS, Lk], F16, name=f"kfT3_{b}") for b in range(BPC)
            ]
            acc = [
                featp.tile([128, 2, 387], F32, name=f"acc_{b}") for b in range(BPC)
            ]
            oacc = [
                featp.tile([128, CONV_G, D], F32, name=f"oacc_{b}") for b in range(BPC)
            ]
            for b in range(BPC):
                nc.vector.memset(kf3[b][:, :, :, 128:129], 1.0)

            # all selection matrices are built during the library-load window
            # (DVE is idle then): s_g[:, pb, span] = (seg[:, g, pb] == span)
            s_tiles = []
            for g in range(NG):
                s_g = sp.tile([128, PB, 128], F16, name=f"S_{g}")
                nc.vector.tensor_tensor(
                    out=s_g[:],
                    in0=seg_sb[:, g, :]
                    .rearrange("p (t o) -> p t o", o=1)
                    .to_broadcast([128, PB, 128]),
                    in1=iota[:]
                    .rearrange("p (o d) -> p o d", o=1)
                    .to_broadcast([128, PB, 128]),
                    op=mybir.AluOpType.is_equal,
                )
                s_tiles.append(s_g)

            gt_tiles = [
                gp.tile([128, 8, E3], F16, name=f"gt_{g}") for g in range(NG)
            ]

            qctr = [0]

            def do_group(g):
                t, b, gg = GROUPS[g]
                gt = gt_tiles[g]
                for side, tabs in ((0, tab_lo), (1, tab_hi)):
                    col = (g * 2 + side) * 32
                    nc.gpsimd.dma_gather(
                        out_ap=gt[:, side * 4 : side * 4 + 4, :],
                        in_ap=tabs[t][:],
                        idxs_ap=idx_sb[:, col : col + 32],
                        num_idxs=512,
                        num_idxs_reg=512,
                        elem_size=E3,
                        queue_num=qctr[0] % 4,
                    )
                    qctr[0] += 1
                # fold pairs: (0,1)->0, (2,4)->2, (3,5)->3, (6,7)->6
                nc.vector.tensor_add(
                    out=gt[:, 0, :], in0=gt[:, 0, :], in1=gt[:, 1, :]
                )
                nc.vector.tensor_add(
                    out=gt[:, 2:4, :], in0=gt[:, 2:4, :], in1=gt[:, 4:6, :]
                )
                nc.vector.tensor_add(
                    out=gt[:, 6, :], in0=gt[:, 6, :], in1=gt[:, 7, :]
                )
                ps = spanps_p.tile([128, E3], F32, tag="ps", name=f"ps_{g}")
                for i, blk in enumerate(SUM_BLOCKS):
                    nc.tensor.matmul(
                        out=ps[:],
                        lhsT=s_tiles[g][:, i, :],
                        rhs=gt[:, blk, :],
                        start=(i == 0),
                        stop=(i == PB - 1),
                    )
                tp = tp_p.tile([128, HOPS, 128], F16, tag="tp", name=f"tpg_{g}")
                if t == "c":
                    cfeat = convscr.tile([128, E3], F16, tag="cfeat", name=f"cf_{g}")
                    nc.vector.tensor_copy(out=cfeat[:], in_=ps[:])
                    for hop in range(HOPS):
                        nc.tensor.transpose(
                            out=tp[:, hop, :],
                            in_=cfeat[:, hop * 128 : (hop + 1) * 128],
                            identity=ident[:],
                        )
                    nc.vector.tensor_copy(
                        out=cfT3[b][:, :, gg * 128 : (gg + 1) * 128], in_=tp[:]
                    )
                else:
                    nc.vector.tensor_copy(
                        out=kf3[b][:, gg, :, 0:128],
                        in_=ps[:].rearrange("p (h d) -> p h d", h=HOPS),
                    )
                    for hop in range(HOPS):
                        nc.tensor.transpose(
                            out=tp[:, hop, :],
                            in_=kf3[b][:, gg, hop, 0:128],
                            identity=ident[:],
                        )
                    nc.vector.tensor_copy(
                        out=kfT3[b][:, :, gg * 128 : (gg + 1) * 128], in_=tp[:]
                    )

            def do_att_incr(b, kk):
                # six bank-aligned [128,129] PSUM regions at (q*512 + j*129),
                # r = hop*2+gg = q*3+j; col 128 is the ones column (row-sums).
                # Every matmul is its own start+stop group; the cross-KB-block
                # accumulation happens in SBUF (acc).
                part = outps_p.tile([128, 2, 512], F32, tag="part", name=f"pt_{b}_{kk}")
                for hop in range(HOPS):
                    att = attps_p.tile(
                        [128, Lc], F32, tag="att", name=f"att_{b}_{kk}_{hop}"
                    )
                    nc.tensor.matmul(
                        out=att[:],
                        lhsT=kfT3[b][:, hop, kk * 128 : (kk + 1) * 128],
                        rhs=cfT3[b][:, hop, :],
                        start=True,
                        stop=True,
                    )
                    expT = expp.tile(
                        [128, Lc], F16, tag="expT", name=f"exp_{b}_{kk}_{hop}"
                    )
                    nc.scalar.activation(
                        out=expT[:],
                        in_=att[:],
                        func=mybir.ActivationFunctionType.Exp,
                        bias=ebias[:],
                    )
                    for gg in range(CONV_G):
                        r = hop * CONV_G + gg
                        q, j = divmod(r, 3)
                        nc.tensor.matmul(
                            out=part[:, q, j * 129 : (j + 1) * 129],
                            lhsT=expT[:, gg * 128 : (gg + 1) * 128],
                            rhs=kf3[b][:, kk, hop, :],
                            start=True,
                            stop=True,
                        )
                if kk == 0:
                    nc.vector.tensor_copy(out=acc[b][:], in_=part[:, :, 0:387])
                else:
                    nc.vector.tensor_add(
                        out=acc[b][:], in0=acc[b][:], in1=part[:, :, 0:387]
                    )

            def finalize(b):
                av = acc[b][:].rearrange("p q (j r) -> p q j r", j=3)
                rinv = softp.tile([128, 2, 3], F32, tag="rinv", name=f"ri_{b}")
                nc.vector.reciprocal(
                    out=rinv[:],
                    in_=av[:, :, :, 128:129].rearrange("p q j o -> p q (j o)"),
                )
                sc = softp.tile([128, 2, 3, D], F32, tag="sc", name=f"sc_{b}")
                nc.vector.tensor_tensor(
                    out=sc[:],
                    in0=av[:, :, :, 0:128],
                    in1=rinv[:]
                    .rearrange("p q (j o) -> p q j o", o=1)
                    .to_broadcast([128, 2, 3, D]),
                    op=mybir.AluOpType.mult,
                )
                # r = hop*2+gg laid out flat in r order: fold the three hops
                scr = sc[:].rearrange("p q j d -> p (q j) d")
                nc.vector.tensor_add(
                    out=oacc[b][:], in0=scr[:, 0:2, :], in1=scr[:, 2:4, :]
                )
                nc.vector.tensor_add(
                    out=oacc[b][:], in0=oacc[b][:], in1=scr[:, 4:6, :]
                )
                for gg in range(CONV_G):
                    nc.sync.dma_start(
                        out=out_d[b, gg * 128 : (gg + 1) * 128, :],
                        in_=oacc[b][:, gg, :],
                    )

            for b in range(BPC):
                base = b * NG_PER_B
                for gg in range(CONV_G):
                    do_group(base + gg)
                for kk in range(KB_G):
                    do_group(base + CONV_G + kk)
                    do_att_incr(b, kk)
                finalize(b)
    nc.compile()
    return nc


def assemble_output(results):
    out = np.empty((Lc, B, D), np.float32)
    for c in range(NCORES):
        o = results[c]["out"]
        for b in range(BPC):
            out[:, c * BPC + b, :] = o[b]
    return out


def kernel(conv_seqs, kb_arr, C, K):
    meta, in_maps = prepare(conv_seqs, kb_arr, C, K)
    nc = build_nc(meta)
    res = run_bass_kernel_spmd(nc, in_maps, list(range(NCORES))).results
    return assemble_output(res)


# revision 7
# speedup vs baseline: 1.0473x; 1.0473x over previous
"""V7 Trainium2 kernel for nn_EncoderTreeSpanNN — paired gathers, lean PE path.

Gathers: extended dma_gather (normal mode — prepare_only races the idx
upload and the library load in this framework version), 24 x 512-row calls
round-robined over the 4 SWDGE queues, one gather tile per group. SP flips
single_packet to test gen/drain pipelining.

Span reduction: each span's 8 tokens form 4 same-partition pairs (LL/LH/HH
types chosen per span; global lo/hi slot budgets met exactly, zero
padding). Two in-place DVE adds fold 8 gather blocks to 4, halving the
span matmuls and selection matrices vs V3. Selection matrices build during
the ~10us GPSIMD library-load window when DVE is idle.

Attention: row-sums fold into the output matmul via a ones column (kf
tiles 129 wide); per-hop bank-aligned PSUM part regions with per-hop SBUF
accumulation so only the last hop's add is in the dependency tail; exp
biased by 2^-4 for f16 range. Gather order is batch-interleaved so the
attention chains spread across the DMA window and only the two final
(independent) chains sit in the tail.
"""

import sys

sys.path.insert(0, "/opt/trn_rl_repo")

import numpy as np

import concourse.bacc as bacc
import concourse.tile as tile
from concourse import mybir
from concourse.bass_utils import run_bass_kernel_spmd

# problem constants
V, D, HOPS = 50000, 128, 3
B, Lc, Mc = 16, 256, 8
Lk, Mk = 512, 8
NCORES = 8
BPC = B // NCORES
E3 = HOPS * D  # 384 elems per row (hop-interleaved)
LO_MAX = 32768  # lo view = tab[0:32768], idx = t (int16 max)
HI_BASE = 17232  # hi view = tab[17232:50000], idx = t-17232 (max 32767)
CONV_G = Lc // 128  # 2
KB_G = Lk // 128  # 4
NG = BPC * (CONV_G + KB_G)  # 12
PB = 4  # pair-blocks per group
SUM_BLOCKS = [0, 2, 3, 6]  # pair sums land here: (0,1)->0 (2,4)->2 (3,5)->3 (6,7)->6

F32 = mybir.dt.float32
F16 = mybir.dt.float16
I16 = mybir.dt.int16

EXP_BIAS = float(-4.0 * np.log(2.0))  # exp(att)*2^-4 keeps f16 range
SP = True  # dma_gather single_packet

# group order = gather issue order; batches interleaved so att chains spread
GROUPS = [
    ("c", 0, 0), ("c", 0, 1),
    ("c", 1, 0), ("c", 1, 1),
    ("k", 0, 0), ("k", 0, 1),
    ("k", 1, 0), ("k", 1, 1),
    ("k", 0, 2), ("k", 1, 2),
    ("k", 0, 3), ("k", 1, 3),
]


def _pack_idx(flat):
    """[n] int16 -> [128, n//16] dma_gather index layout (8 replicas x 16)."""
    n = flat.shape[0]
    return np.tile(flat.reshape(n // 16, 16).T.astype(np.int16), (8, 1))


def _plan_group(toks):
    """toks [128, 8] -> (idx_lo [512], idx_hi [512], seg [128, 4] f32).

    Gather position b*128+p lands in partition p, block b (blocks 0-3 lo,
    4-7 hi). Span k's 8 tokens form 4 pairs typed LL/LH/HH; each pair's two
    tokens share a partition across its pair's two gather blocks, so two DVE
    adds produce one summed row per pair-slot. Every slot is filled.
    """
    c = (toks < HI_BASE).sum(1)
    d = (toks >= LO_MAX).sum(1)
    f = 8 - c - d
    need = 512 - int(c.sum())
    assert 0 <= need <= int(f.sum()), "lo/hi flex band cannot balance group"
    cumf = np.cumsum(f) - f
    cp = c + np.clip(need - cumf, 0, f)  # tokens sent to the lo view, per span
    ymin = cp % 2
    cap = (np.minimum(cp, 8 - cp) - ymin) // 2
    bumps = 256 - int(ymin.sum())
    assert bumps >= 0 and bumps % 2 == 0
    bumps //= 2
    assert cap.sum() >= bumps, "cannot meet LH pair budget"
    cumcap = np.cumsum(cap) - cap
    y = ymin + 2 * np.clip(bumps - cumcap, 0, cap)  # LH pairs per span
    x = (cp - y) // 2  # LL pairs
    z = (8 - cp - y) // 2  # HH pairs
    assert x.sum() == 128 and y.sum() == 256 and z.sum() == 128

    idx_lo = np.empty(512, np.int16)
    idx_hi = np.empty(512, np.int16)
    seg = np.empty((128, PB), np.float32)
    ll = lh = hh = 0
    for k in range(128):
        t = toks[k]
        lo_list = list(t[t < HI_BASE])
        flex_list = list(t[(t >= HI_BASE) & (t < LO_MAX)])
        hi_list = list(t[t >= LO_MAX])
        extra = int(cp[k]) - len(lo_list)
        lo_list += flex_list[:extra]
        hi_list += flex_list[extra:]
        for _ in range(int(y[k])):
            a, bb = lo_list.pop(), hi_list.pop()
            p, slot = (lh, 1) if lh < 128 else (lh - 128, 2)
            lh += 1
            idx_lo[(slot + 1) * 128 + p] = a  # lo blocks 2 / 3
            idx_hi[(slot - 1) * 128 + p] = bb - HI_BASE  # hi blocks 0 / 1
            seg[p, slot] = k
        for _ in range(int(x[k])):
            idx_lo[0 * 128 + ll] = lo_list.pop()
            idx_lo[1 * 128 + ll] = lo_list.pop()
            seg[ll, 0] = k
            ll += 1
        for _ in range(int(z[k])):
            idx_hi[2 * 128 + hh] = hi_list.pop() - HI_BASE
            idx_hi[3 * 128 + hh] = hi_list.pop() - HI_BASE
            seg[hh, 3] = k
            hh += 1
        assert not lo_list and not hi_list
    assert ll == 128 and lh == 256 and hh == 128
    return idx_lo, idx_hi, seg


def prepare(conv_seqs, kb_arr, C, K):
    conv_seqs = np.asarray(conv_seqs)
    kb_arr = np.asarray(kb_arr)

    def row_table(T):
        # [HOPS, V, D] -> [V, HOPS*D] f16 (hop-interleaved rows)
        return (
            np.transpose(np.asarray(T, np.float32), (1, 0, 2))
            .reshape(V, E3)
            .astype(np.float16)
        )

    tab_c = row_table(C)
    tab_k = row_table(K)

    in_maps = []
    for cix in range(NCORES):
        idx_all = np.empty((128, NG * 2 * 32), np.int16)
        seg_all = np.empty((128, NG, PB), np.float32)
        for g, (t, b, gg) in enumerate(GROUPS):
            seqs = conv_seqs if t == "c" else kb_arr
            toks = seqs[cix * BPC + b, gg * 128 : (gg + 1) * 128, :].astype(np.int64)
            idx_lo, idx_hi, seg = _plan_group(toks)
            idx_all[:, (g * 2) * 32 : (g * 2 + 1) * 32] = _pack_idx(idx_lo)
            idx_all[:, (g * 2 + 1) * 32 : (g * 2 + 2) * 32] = _pack_idx(idx_hi)
            seg_all[:, g, :] = seg
        in_maps.append(
            {
                "tab_c": tab_c,
                "tab_k": tab_k,
                "idx_all": idx_all,
                "seg_all": seg_all,
                "ident": np.eye(128, dtype=np.float16),
                "iota": np.broadcast_to(
                    np.arange(128, dtype=np.float32), (128, 128)
                ).copy(),
            }
        )
    return {}, in_maps


def build_nc(meta):
    nc = bacc.Bacc(num_swdge_queues=4)
    tab_c = nc.declare_dram_parameter("tab_c", [V, E3], F16, False)
    tab_k = nc.declare_dram_parameter("tab_k", [V, E3], F16, False)
    idx_d = nc.declare_dram_parameter("idx_all", [128, NG * 2 * 32], I16, False)
    seg_d = nc.declare_dram_parameter("seg_all", [128, NG, PB], F32, False)
    ident_d = nc.declare_dram_parameter("ident", [128, 128], F16, False)
    iota_d = nc.declare_dram_parameter("iota", [128, 128], F32, False)
    out_d = nc.declare_dram_parameter("out", [BPC, Lc, D], F32, True)

    tab_lo = {"c": tab_c[0:LO_MAX], "k": tab_k[0:LO_MAX]}
    tab_hi = {"c": tab_c[HI_BASE:V], "k": tab_k[HI_BASE:V]}

    with tile.TileContext(nc) as tc:
        with (
            tc.tile_pool(name="constp", bufs=1) as constp,
            tc.tile_pool(name="gp", bufs=1) as gp,
            tc.tile_pool(name="sp", bufs=1) as sp,
            tc.tile_pool(name="featp", bufs=1) as featp,
            tc.tile_pool(name="convscr", bufs=2) as convscr,
            tc.tile_pool(name="expp", bufs=3) as expp,
            tc.tile_pool(name="softp", bufs=2) as softp,
            tc.tile_pool(name="outps_p", bufs=1, space="PSUM") as outps_p,
            tc.tile_pool(name="spanps_p", bufs=2, space="PSUM") as spanps_p,
            tc.tile_pool(name="tp_p", bufs=1, space="PSUM") as tp_p,
            tc.tile_pool(name="attps_p", bufs=2, space="PSUM") as attps_p,
        ):
            # tiny dummy gather first: forces the GPSIMD library load to start
            # as early as the engine allows
            dummy_idx = constp.tile([128, 1], I16)
            nc.vector.memset(dummy_idx[:], 0)
            dummy_out = constp.tile([128, 1, E3], F16)
            nc.gpsimd.dma_gather(
                out_ap=dummy_out[:],
                in_ap=tab_c[0:LO_MAX],
                idxs_ap=dummy_idx[:],
                num_idxs=16,
                num_idxs_reg=16,
                elem_size=E3,
                queue_num=0,
                single_packet=SP,
            )
            idx_sb = constp.tile([128, NG * 2 * 32], I16)
            nc.sync.dma_start(out=idx_sb[:], in_=idx_d[:])
            seg_sb = constp.tile([128, NG, PB], F32)
            nc.sync.dma_start(out=seg_sb[:], in_=seg_d[:])
            ident = constp.tile([128, 128], F16)
            nc.sync.dma_start(out=ident[:], in_=ident_d[:])
            iota = constp.tile([128, 128], F32)
            nc.sync.dma_start(out=iota[:], in_=iota_d[:])
            ebias = constp.tile([128, 1], F32)
            nc.vector.memset(ebias[:], EXP_BIAS)

            gt_tiles = [
                gp.tile([128, 8, E3], F16, name=f"gt_{g}") for g in range(NG)
            ]

            # all gathers issued up front, round-robin over the 4 SWDGE queues
            for g, (t, b, gg) in enumerate(GROUPS):
                for side, tabs in ((0, tab_lo), (1, tab_hi)):
                    col = (g * 2 + side) * 32
                    nc.gpsimd.dma_gather(
                        out_ap=gt_tiles[g][:, side * 4 : side * 4 + 4, :],
                        in_ap=tabs[t][:],
                        idxs_ap=idx_sb[:, col : col + 32],
                        num_idxs=512,
                        num_idxs_reg=512,
                        elem_size=E3,
                        queue_num=(2 * g + side) % 4,
                        single_packet=SP,
                    )

            # per-batch persistent feature tiles
            kf3 = [
                featp.tile([128, KB_G, HOPS, 129], F16, name=f"kf3_{b}")
                for b in range(BPC)
            ]
            cfT3 = [
                featp.tile([128, HOPS, Lc], F16, name=f"cfT3_{b}") for b in range(BPC)
            ]
            kfT3 = [
                featp.tile([128, HOPS, Lk], F16, name=f"kfT3_{b}") for b in range(BPC)
            ]
            acc = [
                featp.tile([128, HOPS, 258], F32, name=f"acc_{b}") for b in range(BPC)
            ]
            oacc = [
                featp.tile([128, CONV_G, D], F32, name=f"oacc_{b}") for b in range(BPC)
            ]
            for b in range(BPC):
                nc.vector.memset(kf3[b][:, :, :, 128:129], 1.0)

            # selection matrices built during the library-load dead window:
            # s_g[:, pb, span] = (seg[:, g, pb] == span)
            s_tiles = []
            for g in range(NG):
                s_g = sp.tile([128, PB, 128], F16, name=f"S_{g}")
                nc.vector.tensor_tensor(
                    out=s_g[:],
                    in0=seg_sb[:, g, :]
                    .rearrange("p (t o) -> p t o", o=1)
                    .to_broadcast([128, PB, 128]),
                    in1=iota[:]
                    .rearrange("p (o d) -> p o d", o=1)
                    .to_broadcast([128, PB, 128]),
                    op=mybir.AluOpType.is_equal,
                )
                s_tiles.append(s_g)

            def do_group(g):
                t, b, gg = GROUPS[g]
                gt = gt_tiles[g]
                # fold pairs: (0,1)->0, (2,4)->2, (3,5)->3, (6,7)->6
                nc.vector.tensor_add(
                    out=gt[:, 0, :], in0=gt[:, 0, :], in1=gt[:, 1, :]
                )
                nc.vector.tensor_add(
                    out=gt[:, 2:4, :], in0=gt[:, 2:4, :], in1=gt[:, 4:6, :]
                )
                nc.vector.tensor_add(
                    out=gt[:, 6, :], in0=gt[:, 6, :], in1=gt[:, 7, :]
                )
                ps = spanps_p.tile([128, E3], F32, tag="ps", name=f"ps_{g}")
                for i, blk in enumerate(SUM_BLOCKS):
                    nc.tensor.matmul(
                        out=ps[:],
                        lhsT=s_tiles[g][:, i, :],
                        rhs=gt[:, blk, :],
                        start=(i == 0),
                        stop=(i == PB - 1),
                    )
                tp = tp_p.tile([128, HOPS, 128], F16, tag="tp", name=f"tpg_{g}")
                if t == "c":
                    cfeat = convscr.tile([128, E3], F16, tag="cfeat", name=f"cf_{g}")
                    nc.vector.tensor_copy(out=cfeat[:], in_=ps[:])
                    for hop in range(HOPS):
                        nc.tensor.transpose(
                            out=tp[:, hop, :],
                            in_=cfeat[:, hop * 128 : (hop + 1) * 128],
                            identity=ident[:],
                        )
                    nc.vector.tensor_copy(
                        out=cfT3[b][:, :, gg * 128 : (gg + 1) * 128], in_=tp[:]
                    )
                else:
                    nc.vector.tensor_copy(
                        out=kf3[b][:, gg, :, 0:128],
                        in_=ps[:].rearrange("p (h d) -> p h d", h=HOPS),
                    )
                    for hop in range(HOPS):
                        nc.tensor.transpose(
                            out=tp[:, hop, :],
                            in_=kf3[b][:, gg, hop, 0:128],
                            identity=ident[:],
                        )
                    nc.vector.tensor_copy(
                        out=kfT3[b][:, :, gg * 128 : (gg + 1) * 128], in_=tp[:]
                    )

            def do_att_incr(b, kk):
                # part[:, hop, gg*129:(gg+1)*129]: bank-aligned per hop; col
                # 128 of each region is the ones column (row-sums). Every
                # matmul is its own start+stop group; acc accumulates per-hop
                # on DVE so only the last hop's add is in the tail.
                part = outps_p.tile(
                    [128, HOPS, 512], F32, tag="part", name=f"pt_{b}_{kk}"
                )
                for hop in range(HOPS):
                    att = attps_p.tile(
                        [128, Lc], F32, tag="att", name=f"att_{b}_{kk}_{hop}"
                    )
                    nc.tensor.matmul(
                        out=att[:],
                        lhsT=kfT3[b][:, hop, kk * 128 : (kk + 1) * 128],
                        rhs=cfT3[b][:, hop, :],
                        start=True,
                        stop=True,
                    )
                    expT = expp.tile(
                        [128, Lc], F16, tag="expT", name=f"exp_{b}_{kk}_{hop}"
                    )
                    nc.scalar.activation(
                        out=expT[:],
                        in_=att[:],
                        func=mybir.ActivationFunctionType.Exp,
                        bias=ebias[:],
                    )
                    for gg in range(CONV_G):
                        nc.tensor.matmul(
                            out=part[:, hop, gg * 129 : (gg + 1) * 129],
                            lhsT=expT[:, gg * 128 : (gg + 1) * 128],
                            rhs=kf3[b][:, kk, hop, :],
                            start=True,
                            stop=True,
                        )
                    if kk == 0:
                        nc.vector.tensor_copy(
                            out=acc[b][:, hop, :], in_=part[:, hop, 0:258]
                        )
                    else:
                        nc.vector.tensor_add(
                            out=acc[b][:, hop, :],
                            in0=acc[b][:, hop, :],
                            in1=part[:, hop, 0:258],
                        )

            def finalize(b):
                av = acc[b][:].rearrange("p h (g r) -> p h g r", g=CONV_G)
                rinv = softp.tile([128, HOPS, CONV_G], F32, tag="rinv", name=f"ri_{b}")
                nc.vector.reciprocal(
                    out=rinv[:],
                    in_=av[:, :, :, 128:129].rearrange("p h g o -> p h (g o)"),
                )
                sc = softp.tile([128, HOPS, CONV_G, D], F32, tag="sc", name=f"sc_{b}")
                nc.vector.tensor_tensor(
                    out=sc[:],
                    in0=av[:, :, :, 0:128],
                    in1=rinv[:]
                    .rearrange("p h (g o) -> p h g o", o=1)
                    .to_broadcast([128, HOPS, CONV_G, D]),
                    op=mybir.AluOpType.mult,
                )
                scr = sc[:].rearrange("p h g d -> p (h g) d")
                nc.vector.tensor_add(
                    out=oacc[b][:], in0=scr[:, 0:2, :], in1=scr[:, 2:4, :]
                )
                nc.vector.tensor_add(
                    out=oacc[b][:], in0=oacc[b][:], in1=scr[:, 4:6, :]
                )
                for gg in range(CONV_G):
                    nc.sync.dma_start(
                        out=out_d[b, gg * 128 : (gg + 1) * 128, :],
                        in_=oacc[b][:, gg, :],
                    )

            kb_seen = [0] * BPC
            for g, (t, b, gg) in enumerate(GROUPS):
                do_group(g)
                if t == "k":
                    do_att_incr(b, gg)
                    kb_seen[b] += 1
                    if kb_seen[b] == KB_G:
                        finalize(b)
    nc.compile()
    return nc


def assemble_output(results):
    out = np.empty((Lc, B, D), np.float32)
    for c in range(NCORES):
        o = results[c]["out"]
        for b in range(BPC):
            out[:, c * BPC + b, :] = o[b]
    return out


def kernel(conv_seqs, kb_arr, C, K):
    meta, in_maps = prepare(conv_seqs, kb_arr, C, K)
    nc = build_nc(meta)
    res = run_bass_kernel_spmd(nc, in_maps, list(range(NCORES))).results
    return assemble_output(res)
